# revision 50
# baseline (speedup 1.0000x reference)
"""Causal self-attention (B=4, S=2048, D=1024, H=16) on 8 TRN2 NeuronCores.

Sharding: batch 4-way x head-group 2-way. Core c handles batch c//2 and
heads (c%2)*8 .. (c%2)*8+8. Each core computes its QKV projection slice,
per-head causal attention, and a partial output projection (W_out rows of
its heads); the host sums the two head-group partials per batch.

v2: bf16 inputs/intermediates (fp32 PSUM accumulation), single merged
loop over 512-row seq blocks so projection / attention / out-projection
pipeline across engines, PSUM tag budget sized to let all phases coexist.
"""

import os
import sys

for _p in ("/opt/trn_rl_repo", "/root/.axon_site/_ro/trn_rl_repo"):
    if os.path.isdir(_p) and _p not in sys.path:
        sys.path.insert(0, _p)

import numpy as np

import concourse.bass as bass  # noqa: E402
import concourse.tile as tile  # noqa: E402
from concourse import bacc, mybir  # noqa: E402
from concourse.bass_utils import run_bass_kernel_spmd  # noqa: E402
from concourse.dve_ops import RECIPROCAL_APPROX_FAST, RECIP_APPROX_FAST_CONSTS  # noqa: E402

F32 = mybir.dt.float32
BF16 = mybir.dt.bfloat16

B = 4
S = 2048
D = 1024
H = 16
HD = 64  # head dim
HLOC = 8  # heads per core
SCALE = HD ** -0.5

C = D // 128  # dm chunks (8)
NQS = S // 512  # seq blocks (4)
NST = S // 128  # 128-row seq tiles (16)
NT = HLOC * HD // 128  # Q/K partition groups (4)
NFC = NT  # vals f-chunks (4)


def _build_nc(use_bias: bool, repeat: int = 1):
    from contextlib import ExitStack

    nc = bacc.Bacc(
        "TRN2",
        target_bir_lowering=False,
        debug=False,
        enable_asserts=True,
        num_devices=8,
    )

    dw = D + 1 if use_bias else D
    CW = C + 1 if use_bias else C
    xT = nc.dram_tensor("xT", [dw, S], BF16, kind="ExternalInput")
    wq = nc.dram_tensor("wq", [dw, 512], BF16, kind="ExternalInput")
    wk = nc.dram_tensor("wk", [dw, 512], BF16, kind="ExternalInput")
    wv = nc.dram_tensor("wv", [dw, 512], BF16, kind="ExternalInput")
    wo = nc.dram_tensor("wo", [512, D], BF16, kind="ExternalInput")
    mask = nc.dram_tensor("mask", [128, 128], BF16, kind="ExternalInput")
    y = nc.dram_tensor("y", [S, D], F32, kind="ExternalOutput")

    with tile.TileContext(nc) as tc, ExitStack() as es:
        if repeat > 1:
            # huge body (>256 instrs/engine): hint the back-edge target so
            # the branch I$-hits instead of stalling ~4us on an IRAM fetch
            es.enter_context(
                tc.For_i(
                    0, repeat, 1,
                    hint_engines=(
                        mybir.EngineType.PE,
                        mybir.EngineType.DVE,
                        mybir.EngineType.Activation,
                    ),
                )
            )
        with tc.tile_pool(name="persist", bufs=1) as persist:
            mask_sb = persist.tile([128, 128], BF16)
            QT_sb = persist.tile([128, NT, S], BF16)
            KT_sb = persist.tile([128, NT, S], BF16)
            V_sb = persist.tile([128, NST, HLOC, HD + 1], BF16)
            vals_sb = persist.tile([128, NFC, S], BF16)
            # only the ones-plane (column HD of each head block) must be 1.0;
            # the value columns are fully overwritten by the vp copies
            nc.vector.memset(V_sb[:, :, :, HD : HD + 1], 1.0)

            wq_sb = persist.tile([128, CW, 512], BF16)
            wk_sb = persist.tile([128, CW, 512], BF16)
            wv_sb = persist.tile([128, CW, 512], BF16)
            wo_sb = persist.tile([128, NFC, D], BF16)

            # DMA descriptor generation and the transfer pipe both serialize,
            # so issue few, big transfers on ONE ring in consumption order:
            # wq/x first (Q chains), then wk, wv, mask. wo rides the gpsimd
            # ring; it isn't needed until the first out-projection.
            def _load_w(eng, wsb, wdr, half, split=2):
                cs = C // split
                for h in ([half] if half is not None else range(split)):
                    eng.dma_start(
                        wsb[:, h * cs : (h + 1) * cs, :],
                        wdr[h * cs * 128 : (h + 1) * cs * 128, :].rearrange(
                            "(c p) f -> p c f", p=128
                        ),
                    )
                if use_bias and (half is None or half == split - 1):
                    eng.dma_start(wsb[0:1, C, :], wdr[D : D + 1, :])

            nc.gpsimd.dma_start(
                wo_sb[:], wo.rearrange("(c p) f -> p c f", p=128)
            )

            with (
                tc.tile_pool(name="work", bufs=1) as work,
                tc.tile_pool(name="ps", bufs=1, space="PSUM") as ps,
            ):
                def p1_xq(qs):
                    """xq tile + input DMAs for seq block qs."""
                    sq = slice(qs * 512, (qs + 1) * 512)
                    xq = work.tile([128, CW, 512], BF16, tag="xq", bufs=2,
                                   name="xq")
                    if qs == 0:
                        # interleave wq quarters with x quarters in consumption
                        # order; wk halves, then wv/mask queue behind them
                        for h in range(4):
                            cs = C // 4
                            _load_w(nc.sync, wq_sb, wq, h, split=4)
                            nc.sync.dma_start(
                                xq[:, h * cs : (h + 1) * cs, :],
                                xT[h * cs * 128 : (h + 1) * cs * 128, sq].rearrange(
                                    "(c p) s -> p c s", p=128
                                ),
                            )
                        _load_w(nc.sync, wk_sb, wk, 0)
                        _load_w(nc.sync, wk_sb, wk, 1)
                        _load_w(nc.sync, wv_sb, wv, None)
                        nc.sync.dma_start(mask_sb[:], mask[:])
                    else:
                        nc.sync.dma_start(
                            xq[:, 0:C, :],
                            xT[0:D, sq].rearrange("(c p) s -> p c s", p=128),
                        )
                    if use_bias:
                        nc.vector.memset(xq[0:1, C, :], 1.0)
                    return xq

                def p1_qk(qs, xq, wsb, dst, t):
                    """one Q^T or K^T projection chain (t-group) of block qs"""
                    sq = slice(qs * 512, (qs + 1) * 512)
                    qp = ps.tile([128, 512], F32, tag="p1_ps", bufs=2, name="qp")
                    for c in range(C):
                        nc.tensor.matmul(
                            qp[:],
                            lhsT=wsb[:, c, t * 128 : (t + 1) * 128],
                            rhs=xq[:, c, :],
                            start=(c == 0),
                            stop=(c == C - 1 and not use_bias),
                        )
                    if use_bias:
                        nc.tensor.matmul(
                            qp[:],
                            lhsT=wsb[0:1, C, t * 128 : (t + 1) * 128],
                            rhs=xq[0:1, C, :],
                            start=False,
                            stop=True,
                        )
                    nc.vector.tensor_copy(dst[:, t, sq], qp[:])

                def p1_v(qs, xq, sst):
                    """one V projection chain (128 k-positions) of block qs"""
                    st = qs * 4 + sst
                    sl = slice(sst * 128, (sst + 1) * 128)
                    vp = ps.tile([128, 512], F32, tag="p1_ps", bufs=2, name="vp")
                    for c in range(C):
                        nc.tensor.matmul(
                            vp[:],
                            lhsT=xq[:, c, sl],
                            rhs=wv_sb[:, c, :],
                            start=(c == 0),
                            stop=(c == C - 1 and not use_bias),
                        )
                    if use_bias:
                        nc.tensor.matmul(
                            vp[:],
                            lhsT=xq[0:1, C, sl],
                            rhs=wv_sb[0:1, C, :],
                            start=False,
                            stop=True,
                        )
                    nc.vector.tensor_copy(
                        V_sb[:, st, :, 0:HD],
                        vp.rearrange("p (h e) -> p h e", h=HLOC),
                    )

                def p1_block(qs, xq):
                    for wsb, dst in ((wq_sb, QT_sb), (wk_sb, KT_sb)):
                        for t in range(NT):
                            p1_qk(qs, xq, wsb, dst, t)
                    for sst in range(4):
                        p1_v(qs, xq, sst)

                # prologue: projections for block 0
                xq_cur = p1_xq(0)
                p1_block(0, xq_cur)

                for qs in range(NQS):
                    sq = slice(qs * 512, (qs + 1) * 512)
                    xq_nxt = p1_xq(qs + 1) if qs + 1 < NQS else None

                    # ---- P2(qs): causal attention for this q block ----
                    # head pair (2t, 2t+1): even head on partitions 0-63, odd
                    # on 64-127 -> adjacent K=64 score matmuls target disjoint
                    # PE row groups and can run concurrently.
                    for t in range(NT):
                        outs = [
                            ps.tile([HD + 1, 512], F32, tag="outy", bufs=2,
                                    name=f"out{p}")
                            for p in range(2)
                        ]
                        # diagonal k-chunks first, then 0..4qs-1
                        js = list(range(4 * qs, 4 * qs + 4)) + list(range(0, 4 * qs))
                        npos = len(js)
                        for gb, j in enumerate(js):
                            dg = j - 4 * qs
                            qlo = dg * 128 if 0 <= dg < 4 else 0
                            # head-major score tile: one exp covers both heads,
                            # trimmed to the causally-valid columns
                            sc = ps.tile([128, 2, 512], F32, tag="sc", bufs=2,
                                         name="sc")
                            for p in range(2):
                                po = p * HD
                                nc.tensor.matmul(
                                    sc[:, p, qlo:512],
                                    lhsT=KT_sb[po : po + HD, t,
                                               j * 128 : (j + 1) * 128],
                                    rhs=QT_sb[po : po + HD, t,
                                              qs * 512 + qlo : (qs + 1) * 512],
                                    start=True,
                                    stop=True,
                                )
                            ex = work.tile([128, 2, 512], BF16, tag="ex",
                                           bufs=4, name="ex")
                            nc.scalar.activation(
                                ex[:, :, qlo:512],
                                sc[:, :, qlo:512],
                                mybir.ActivationFunctionType.Exp, scale=SCALE,
                            )
                            if 0 <= dg < 4:
                                # mask only the 128-wide mixed band; columns
                                # < 128*dg are excluded from the PV moving
                                # range below (exact zeros).
                                for p in range(2):
                                    nc.vector.tensor_mul(
                                        ex[:, p, dg * 128 : (dg + 1) * 128],
                                        ex[:, p, dg * 128 : (dg + 1) * 128],
                                        mask_sb[:],
                                    )
                            for p in range(2):
                                nc.tensor.matmul(
                                    outs[p][:, qlo:512],
                                    lhsT=V_sb[:, j, 2 * t + p, :],
                                    rhs=ex[:, p, qlo:512],
                                    start=(gb == 0),
                                    stop=(gb == npos - 1),
                                )
                        # normalize: rows 0..63 divided by row 64. One copy
                        # evacuates the PSUM accumulator (freeing its bank for
                        # the next head pair / out-projection); the divide
                        # works from the SBUF copy.
                        for p in range(2):
                            po = p * HD
                            r_row = work.tile([1, 512], F32, tag=f"r_row{p}", bufs=2,
                                              name=f"r_row{p}")
                            nc.vector.tensor_copy(r_row[:], outs[p][HD : HD + 1, :])
                            nc.vector._custom_dve(
                                RECIPROCAL_APPROX_FAST,
                                out=r_row[:],
                                in0=r_row[:],
                                s0=RECIP_APPROX_FAST_CONSTS["s0"],
                                s1=RECIP_APPROX_FAST_CONSTS["s1"],
                                imm2=RECIP_APPROX_FAST_CONSTS["imm2"],
                            )
                            rc = work.tile([HD, 512], F32, tag=f"rc{p}", bufs=2,
                                           name=f"rc{p}")
                            nc.gpsimd.partition_broadcast(rc[:], r_row[:])
                            nc.vector.tensor_mul(
                                vals_sb[po : po + HD, t, sq], outs[p][0:HD, :], rc[:]
                            )

                        # weave next block's projections between attention
                        # t-iterations: ready PE filler sits right next to
                        # every exp-wait stall in the static schedule
                        if xq_nxt is not None:
                            if t < 2:
                                p1_qk(qs + 1, xq_nxt, wq_sb, QT_sb, 2 * t)
                                p1_qk(qs + 1, xq_nxt, wq_sb, QT_sb, 2 * t + 1)
                            else:
                                p1_qk(qs + 1, xq_nxt, wk_sb, KT_sb, 2 * (t - 2))
                                p1_qk(qs + 1, xq_nxt, wk_sb, KT_sb, 2 * (t - 2) + 1)
                                p1_v(qs + 1, xq_nxt, 2 * (t - 2))
                                p1_v(qs + 1, xq_nxt, 2 * (t - 2) + 1)

                    xq_cur = xq_nxt

                    # ---- P3(qs): output projection for this seq block ----
                    for sst in range(4):
                        st = qs * 4 + sst
                        sl = slice(st * 128, (st + 1) * 128)
                        for nh in range(2):
                            hs = slice(nh * 512, (nh + 1) * 512)
                            yp = ps.tile([128, 512], F32, tag="outy", bufs=2,
                                         name="yp")
                            for fc in range(NFC):
                                nc.tensor.matmul(
                                    yp[:],
                                    lhsT=vals_sb[:, fc, sl],
                                    rhs=wo_sb[:, fc, hs],
                                    start=(fc == 0),
                                    stop=(fc == NFC - 1),
                                )
                            yo = work.tile([128, 512], F32, tag="yo", bufs=3)
                            nc.vector.tensor_copy(yo[:], yp[:])
                            nc.gpsimd.dma_start(y[sl, hs], yo[:])

    nc.finalize()
    return nc


_NC_CACHE = {}


def _get_nc(use_bias: bool, repeat: int = 1):
    key = (use_bias, repeat)
    if key not in _NC_CACHE:
        _NC_CACHE[key] = _build_nc(use_bias, repeat)
    return _NC_CACHE[key]


def _make_mask() -> np.ndarray:
    # upper-tri-inclusive band mask: keep[k_local, q_local] = q_local >= k_local
    kl = np.arange(128)[:, None]
    ql = np.arange(128)[None, :]
    return (ql >= kl).astype(np.float32)


def _bf16(a: np.ndarray) -> np.ndarray:
    import ml_dtypes

    return np.ascontiguousarray(a).astype(ml_dtypes.bfloat16)


def make_in_maps(x, W_qkv, b_qkv, W_out):
    use_bias = bool(np.any(b_qkv))
    mask = _bf16(_make_mask())
    in_maps = []
    for core in range(8):
        b = core // 2
        hg = core % 2
        xt = np.ascontiguousarray(x[b].T)  # [D, S]
        q_cols = slice(hg * 512, (hg + 1) * 512)
        k_cols = slice(D + hg * 512, D + (hg + 1) * 512)
        v_cols = slice(2 * D + hg * 512, 2 * D + (hg + 1) * 512)
        wq_s = np.ascontiguousarray(W_qkv[:, q_cols])
        wk_s = np.ascontiguousarray(W_qkv[:, k_cols])
        wv_s = np.ascontiguousarray(W_qkv[:, v_cols])
        if use_bias:
            xt = np.concatenate([xt, np.ones((1, S), np.float32)], axis=0)
            wq_s = np.concatenate([wq_s, b_qkv[None, hg * 512 : (hg + 1) * 512]], axis=0)
            wk_s = np.concatenate(
                [wk_s, b_qkv[None, D + hg * 512 : D + (hg + 1) * 512]], axis=0
            )
            wv_s = np.concatenate(
                [wv_s, b_qkv[None, 2 * D + hg * 512 : 2 * D + (hg + 1) * 512]], axis=0
            )
        wo_s = np.ascontiguousarray(W_out[hg * 512 : (hg + 1) * 512, :])
        in_maps.append(
            {
                "xT": _bf16(xt),
                "wq": _bf16(wq_s),
                "wk": _bf16(wk_s),
                "wv": _bf16(wv_s),
                "wo": _bf16(wo_s),
                "mask": mask,
            }
        )
    return in_maps, use_bias


def gather_output(results, b_out):
    y = np.empty((B, S, D), dtype=np.float32)
    for b in range(B):
        y[b] = results[2 * b]["y"] + results[2 * b + 1]["y"]
    if b_out is not None and np.any(b_out):
        y += b_out[None, None, :].astype(np.float32)
    return y


def kernel(x, W_qkv, b_qkv, W_out, b_out):
    x = np.asarray(x, dtype=np.float32)
    W_qkv = np.asarray(W_qkv, dtype=np.float32)
    b_qkv = np.asarray(b_qkv, dtype=np.float32)
    W_out = np.asarray(W_out, dtype=np.float32)
    b_out = np.asarray(b_out, dtype=np.float32)
    in_maps, use_bias = make_in_maps(x, W_qkv, b_qkv, W_out)
    nc = _get_nc(use_bias)
    res = run_bass_kernel_spmd(nc, in_maps, core_ids=list(range(8)))
    return gather_output(res.results, b_out)


# revision 53
# speedup vs baseline: 1.0988x; 1.0988x over previous
"""Causal self-attention (B=4, S=2048, D=1024, H=16) on 8 TRN2 NeuronCores.

Sharding: batch 4-way x head-group 2-way. Core c handles batch c//2 and
heads (c%2)*8 .. (c%2)*8+8. Each core computes its QKV projection slice,
per-head causal attention, and a partial output projection (W_out rows of
its heads); the host sums the two head-group partials per batch.

v2: bf16 inputs/intermediates (fp32 PSUM accumulation), single merged
loop over 512-row seq blocks so projection / attention / out-projection
pipeline across engines, PSUM tag budget sized to let all phases coexist.
"""

import os
import sys

for _p in ("/opt/trn_rl_repo", "/root/.axon_site/_ro/trn_rl_repo"):
    if os.path.isdir(_p) and _p not in sys.path:
        sys.path.insert(0, _p)

import numpy as np

import concourse.bass as bass  # noqa: E402
import concourse.tile as tile  # noqa: E402
from concourse import bacc, mybir  # noqa: E402
from concourse.bass_utils import run_bass_kernel_spmd  # noqa: E402
from concourse.dve_ops import RECIPROCAL_APPROX_FAST, RECIP_APPROX_FAST_CONSTS  # noqa: E402

F32 = mybir.dt.float32
BF16 = mybir.dt.bfloat16

B = 4
S = 2048
D = 1024
H = 16
HD = 64  # head dim
HLOC = 8  # heads per core
SCALE = HD ** -0.5

C = D // 128  # dm chunks (8)
NQS = S // 512  # seq blocks (4)
NST = S // 128  # 128-row seq tiles (16)
NT = HLOC * HD // 128  # Q/K partition groups (4)
NFC = NT  # vals f-chunks (4)


def _build_nc(use_bias: bool, repeat: int = 1):
    from contextlib import ExitStack

    nc = bacc.Bacc(
        "TRN2",
        target_bir_lowering=False,
        debug=False,
        enable_asserts=True,
        num_devices=8,
    )

    dw = D + 1 if use_bias else D
    CW = C + 1 if use_bias else C
    xT = nc.dram_tensor("xT", [dw, S], BF16, kind="ExternalInput")
    wq = nc.dram_tensor("wq", [dw, 512], BF16, kind="ExternalInput")
    wk = nc.dram_tensor("wk", [dw, 512], BF16, kind="ExternalInput")
    wv = nc.dram_tensor("wv", [dw, 512], BF16, kind="ExternalInput")
    wo = nc.dram_tensor("wo", [512, D], BF16, kind="ExternalInput")
    mask = nc.dram_tensor("mask", [128, 128], BF16, kind="ExternalInput")
    y = nc.dram_tensor("y", [S, D], F32, kind="ExternalOutput")

    with tile.TileContext(nc) as tc, ExitStack() as es:
        if repeat > 1:
            # huge body (>256 instrs/engine): hint the back-edge target so
            # the branch I$-hits instead of stalling ~4us on an IRAM fetch
            es.enter_context(
                tc.For_i(
                    0, repeat, 1,
                    hint_engines=(
                        mybir.EngineType.PE,
                        mybir.EngineType.DVE,
                        mybir.EngineType.Activation,
                    ),
                )
            )
        with tc.tile_pool(name="persist", bufs=1) as persist:
            mask_sb = persist.tile([128, 128], BF16)
            QT_sb = persist.tile([128, NT, S], BF16)
            KT_sb = persist.tile([128, NT, S], BF16)
            V_sb = persist.tile([128, NST, HLOC, HD + 1], BF16)
            vals_sb = persist.tile([128, NFC, S], BF16)
            # only the ones-plane (column HD of each head block) must be 1.0;
            # the value columns are fully overwritten by the vp copies
            nc.vector.memset(V_sb[:, :, :, HD : HD + 1], 1.0)

            wq_sb = persist.tile([128, CW, 512], BF16)
            wk_sb = persist.tile([128, CW, 512], BF16)
            wv_sb = persist.tile([128, CW, 512], BF16)
            wo_sb = persist.tile([128, NFC, D], BF16)

            # DMA descriptor generation and the transfer pipe both serialize,
            # so issue few, big transfers on ONE ring in consumption order:
            # wq/x first (Q chains), then wk, wv, mask. wo rides the gpsimd
            # ring; it isn't needed until the first out-projection.
            def _load_w(eng, wsb, wdr, half, split=2):
                cs = C // split
                for h in ([half] if half is not None else range(split)):
                    eng.dma_start(
                        wsb[:, h * cs : (h + 1) * cs, :],
                        wdr[h * cs * 128 : (h + 1) * cs * 128, :].rearrange(
                            "(c p) f -> p c f", p=128
                        ),
                    )
                if use_bias and (half is None or half == split - 1):
                    eng.dma_start(wsb[0:1, C, :], wdr[D : D + 1, :])

            nc.gpsimd.dma_start(
                wo_sb[:], wo.rearrange("(c p) f -> p c f", p=128)
            )

            with (
                tc.tile_pool(name="work", bufs=1) as work,
                tc.tile_pool(name="ps", bufs=1, space="PSUM") as ps,
            ):
                def p1_xq(qs):
                    """xq tile + input DMAs for seq block qs."""
                    sq = slice(qs * 512, (qs + 1) * 512)
                    xq = work.tile([128, CW, 512], BF16, tag="xq", bufs=2,
                                   name="xq")
                    if qs == 0:
                        # interleave wq quarters with x quarters in consumption
                        # order; wk halves, then wv/mask queue behind them
                        for h in range(4):
                            cs = C // 4
                            _load_w(nc.sync, wq_sb, wq, h, split=4)
                            nc.sync.dma_start(
                                xq[:, h * cs : (h + 1) * cs, :],
                                xT[h * cs * 128 : (h + 1) * cs * 128, sq].rearrange(
                                    "(c p) s -> p c s", p=128
                                ),
                            )
                        _load_w(nc.sync, wk_sb, wk, 0)
                        _load_w(nc.sync, wk_sb, wk, 1)
                        _load_w(nc.sync, wv_sb, wv, None)
                        nc.sync.dma_start(mask_sb[:], mask[:])
                    else:
                        nc.sync.dma_start(
                            xq[:, 0:C, :],
                            xT[0:D, sq].rearrange("(c p) s -> p c s", p=128),
                        )
                    if use_bias:
                        nc.vector.memset(xq[0:1, C, :], 1.0)
                    return xq

                def p1_qk(qs, xq, wsb, dst, t):
                    """one Q^T or K^T projection chain (t-group) of block qs"""
                    sq = slice(qs * 512, (qs + 1) * 512)
                    qp = ps.tile([128, 512], F32, tag="p1_ps", bufs=2, name="qp")
                    for c in range(C):
                        nc.tensor.matmul(
                            qp[:],
                            lhsT=wsb[:, c, t * 128 : (t + 1) * 128],
                            rhs=xq[:, c, :],
                            start=(c == 0),
                            stop=(c == C - 1 and not use_bias),
                        )
                    if use_bias:
                        nc.tensor.matmul(
                            qp[:],
                            lhsT=wsb[0:1, C, t * 128 : (t + 1) * 128],
                            rhs=xq[0:1, C, :],
                            start=False,
                            stop=True,
                        )
                    nc.vector.tensor_copy(dst[:, t, sq], qp[:])

                def p1_v(qs, xq, sst):
                    """one V projection chain (128 k-positions) of block qs"""
                    st = qs * 4 + sst
                    sl = slice(sst * 128, (sst + 1) * 128)
                    vp = ps.tile([128, 512], F32, tag="p1_ps", bufs=2, name="vp")
                    for c in range(C):
                        nc.tensor.matmul(
                            vp[:],
                            lhsT=xq[:, c, sl],
                            rhs=wv_sb[:, c, :],
                            start=(c == 0),
                            stop=(c == C - 1 and not use_bias),
                        )
                    if use_bias:
                        nc.tensor.matmul(
                            vp[:],
                            lhsT=xq[0:1, C, sl],
                            rhs=wv_sb[0:1, C, :],
                            start=False,
                            stop=True,
                        )
                    nc.vector.tensor_copy(
                        V_sb[:, st, :, 0:HD],
                        vp.rearrange("p (h e) -> p h e", h=HLOC),
                    )

                def p1_block(qs, xq):
                    for wsb, dst in ((wq_sb, QT_sb), (wk_sb, KT_sb)):
                        for t in range(NT):
                            p1_qk(qs, xq, wsb, dst, t)
                    for sst in range(4):
                        p1_v(qs, xq, sst)

                # prologue: projections for block 0
                xq_cur = p1_xq(0)
                p1_block(0, xq_cur)

                for qs in range(NQS):
                    sq = slice(qs * 512, (qs + 1) * 512)
                    xq_nxt = p1_xq(qs + 1) if qs + 1 < NQS else None

                    # ---- P2(qs): causal attention for this q block ----
                    # head pair (2t, 2t+1): even head on partitions 0-63, odd
                    # on 64-127 -> adjacent K=64 score matmuls target disjoint
                    # PE row groups and can run concurrently.
                    for t in range(NT):
                        outs = [
                            ps.tile([HD + 1, 512], F32, tag="outy", bufs=2,
                                    name=f"out{p}")
                            for p in range(2)
                        ]
                        # diagonal k-chunks first, then 0..4qs-1
                        js = list(range(4 * qs, 4 * qs + 4)) + list(range(0, 4 * qs))
                        npos = len(js)
                        for gb, j in enumerate(js):
                            dg = j - 4 * qs
                            qlo = dg * 128 if 0 <= dg < 4 else 0
                            # head-major score tile: one exp covers both heads,
                            # trimmed to the causally-valid columns
                            sc = ps.tile([128, 2, 512], F32, tag="sc", bufs=2,
                                         name="sc")
                            for p in range(2):
                                po = p * HD
                                nc.tensor.matmul(
                                    sc[:, p, qlo:512],
                                    lhsT=KT_sb[po : po + HD, t,
                                               j * 128 : (j + 1) * 128],
                                    rhs=QT_sb[po : po + HD, t,
                                              qs * 512 + qlo : (qs + 1) * 512],
                                    start=True,
                                    stop=True,
                                )
                            ex = work.tile([128, 2, 512], BF16, tag="ex",
                                           bufs=4, name="ex")
                            nc.scalar.activation(
                                ex[:, :, qlo:512],
                                sc[:, :, qlo:512],
                                mybir.ActivationFunctionType.Exp, scale=SCALE,
                            )
                            if 0 <= dg < 4:
                                # mask only the 128-wide mixed band; columns
                                # < 128*dg are excluded from the PV moving
                                # range below (exact zeros).
                                for p in range(2):
                                    nc.vector.tensor_mul(
                                        ex[:, p, dg * 128 : (dg + 1) * 128],
                                        ex[:, p, dg * 128 : (dg + 1) * 128],
                                        mask_sb[:],
                                    )
                            for p in range(2):
                                nc.tensor.matmul(
                                    outs[p][:, qlo:512],
                                    lhsT=V_sb[:, j, 2 * t + p, :],
                                    rhs=ex[:, p, qlo:512],
                                    start=(gb == 0),
                                    stop=(gb == npos - 1),
                                )
                        # normalize: rows 0..63 divided by row 64. One copy
                        # evacuates the PSUM accumulator (freeing its bank for
                        # the next head pair / out-projection); the divide
                        # works from the SBUF copy.
                        for p in range(2):
                            po = p * HD
                            r_row = work.tile([1, 512], F32, tag=f"r_row{p}", bufs=2,
                                              name=f"r_row{p}")
                            nc.vector.tensor_copy(r_row[:], outs[p][HD : HD + 1, :])
                            nc.vector._custom_dve(
                                RECIPROCAL_APPROX_FAST,
                                out=r_row[:],
                                in0=r_row[:],
                                s0=RECIP_APPROX_FAST_CONSTS["s0"],
                                s1=RECIP_APPROX_FAST_CONSTS["s1"],
                                imm2=RECIP_APPROX_FAST_CONSTS["imm2"],
                            )
                            rc = work.tile([HD, 512], F32, tag=f"rc{p}", bufs=2,
                                           name=f"rc{p}")
                            nc.gpsimd.partition_broadcast(rc[:], r_row[:])
                            nc.vector.tensor_mul(
                                vals_sb[po : po + HD, t, sq], outs[p][0:HD, :], rc[:]
                            )

                        # weave next block's projections between attention
                        # t-iterations: ready PE filler sits right next to
                        # every exp-wait stall in the static schedule
                        if xq_nxt is not None:
                            if t < 2:
                                p1_qk(qs + 1, xq_nxt, wq_sb, QT_sb, 2 * t)
                                p1_qk(qs + 1, xq_nxt, wq_sb, QT_sb, 2 * t + 1)
                            else:
                                p1_qk(qs + 1, xq_nxt, wk_sb, KT_sb, 2 * (t - 2))
                                p1_qk(qs + 1, xq_nxt, wk_sb, KT_sb, 2 * (t - 2) + 1)
                                p1_v(qs + 1, xq_nxt, 2 * (t - 2))
                                p1_v(qs + 1, xq_nxt, 2 * (t - 2) + 1)

                    xq_cur = xq_nxt

                    # ---- P3(qs): output projection for this seq block ----
                    for sst in range(4):
                        st = qs * 4 + sst
                        sl = slice(st * 128, (st + 1) * 128)
                        for nh in range(2):
                            hs = slice(nh * 512, (nh + 1) * 512)
                            # last block: score slots are free forever, so
                            # borrow them — yp then needn't wait for the
                            # final head-pair's normalize to release "outy"
                            yp = ps.tile([128, 512], F32,
                                         tag=("sc" if qs == NQS - 1 else "outy"),
                                         bufs=2, name="yp")
                            for fc in range(NFC):
                                nc.tensor.matmul(
                                    yp[:],
                                    lhsT=vals_sb[:, fc, sl],
                                    rhs=wo_sb[:, fc, hs],
                                    start=(fc == 0),
                                    stop=(fc == NFC - 1),
                                )
                            yo = work.tile([128, 512], F32, tag="yo", bufs=3)
                            nc.vector.tensor_copy(yo[:], yp[:])
                            nc.gpsimd.dma_start(y[sl, hs], yo[:])

    nc.finalize()
    return nc


_NC_CACHE = {}


def _get_nc(use_bias: bool, repeat: int = 1):
    key = (use_bias, repeat)
    if key not in _NC_CACHE:
        _NC_CACHE[key] = _build_nc(use_bias, repeat)
    return _NC_CACHE[key]


def _make_mask() -> np.ndarray:
    # upper-tri-inclusive band mask: keep[k_local, q_local] = q_local >= k_local
    kl = np.arange(128)[:, None]
    ql = np.arange(128)[None, :]
    return (ql >= kl).astype(np.float32)


def _bf16(a: np.ndarray) -> np.ndarray:
    import ml_dtypes

    return np.ascontiguousarray(a).astype(ml_dtypes.bfloat16)


def make_in_maps(x, W_qkv, b_qkv, W_out):
    use_bias = bool(np.any(b_qkv))
    mask = _bf16(_make_mask())
    in_maps = []
    for core in range(8):
        b = core // 2
        hg = core % 2
        xt = np.ascontiguousarray(x[b].T)  # [D, S]
        q_cols = slice(hg * 512, (hg + 1) * 512)
        k_cols = slice(D + hg * 512, D + (hg + 1) * 512)
        v_cols = slice(2 * D + hg * 512, 2 * D + (hg + 1) * 512)
        wq_s = np.ascontiguousarray(W_qkv[:, q_cols])
        wk_s = np.ascontiguousarray(W_qkv[:, k_cols])
        wv_s = np.ascontiguousarray(W_qkv[:, v_cols])
        if use_bias:
            xt = np.concatenate([xt, np.ones((1, S), np.float32)], axis=0)
            wq_s = np.concatenate([wq_s, b_qkv[None, hg * 512 : (hg + 1) * 512]], axis=0)
            wk_s = np.concatenate(
                [wk_s, b_qkv[None, D + hg * 512 : D + (hg + 1) * 512]], axis=0
            )
            wv_s = np.concatenate(
                [wv_s, b_qkv[None, 2 * D + hg * 512 : 2 * D + (hg + 1) * 512]], axis=0
            )
        wo_s = np.ascontiguousarray(W_out[hg * 512 : (hg + 1) * 512, :])
        in_maps.append(
            {
                "xT": _bf16(xt),
                "wq": _bf16(wq_s),
                "wk": _bf16(wk_s),
                "wv": _bf16(wv_s),
                "wo": _bf16(wo_s),
                "mask": mask,
            }
        )
    return in_maps, use_bias


def gather_output(results, b_out):
    y = np.empty((B, S, D), dtype=np.float32)
    for b in range(B):
        y[b] = results[2 * b]["y"] + results[2 * b + 1]["y"]
    if b_out is not None and np.any(b_out):
        y += b_out[None, None, :].astype(np.float32)
    return y


def kernel(x, W_qkv, b_qkv, W_out, b_out):
    x = np.asarray(x, dtype=np.float32)
    W_qkv = np.asarray(W_qkv, dtype=np.float32)
    b_qkv = np.asarray(b_qkv, dtype=np.float32)
    W_out = np.asarray(W_out, dtype=np.float32)
    b_out = np.asarray(b_out, dtype=np.float32)
    in_maps, use_bias = make_in_maps(x, W_qkv, b_qkv, W_out)
    nc = _get_nc(use_bias)
    res = run_bass_kernel_spmd(nc, in_maps, core_ids=list(range(8)))
    return gather_output(res.results, b_out)
